# revision 15
# baseline (speedup 1.0000x reference)
"""Trainium2 Bass kernel for nn_EntailmentSelfAttention (8-core data parallel).

Mapping (one n per NeuronCore; S=2 sentences iterated inside):
  - Transposed on-chip layout: head-dim on partitions, sequence on the free
    axis, so the softmax (over queries) reduces along the free axis.
  - q-side projection folded on the HOST: yq = (Wk^T Wq) @ q^T; the on-chip
    energy matmul is raw keys (stationary) x yq (moving), 64-deep contraction.
  - Heads processed in PAIRS packed on SBUF partitions (head A on 0:64, head
    B on 64:128). The two 64-row energy matmuls use tile_position row groups
    (0,0)/(64,0) and run CONCURRENTLY in the PE array; the two attn@v matmuls
    are column-group paired (0,0)/(0,64) into one PSUM bank.
  - Masked query positions dropped on the host (compaction to QP columns).
    Pad columns have yq = 0 -> energy 0 -> exp = 1; the pad count is
    subtracted from the softmax denominators on-chip (npads input).
  - exp on the scalar engine: for DVE_CHUNKS, one ACTIVATE covers both
    chunks of a duo via a 2-bank strided PSUM AP (bf16 out runs 2x) and the
    rowsums come from DVE tensor_scalar+accum_out (in-place); for the other
    chunks, per-chunk ACTIVATE with accum_out so the rowsum rides the ACT
    engine (load balance across ACT/DVE).
  - xv scaling by 1/rowsum: one fused scalar_tensor_tensor per pair with a
    broadcast reciprocal.
  - Software pipeline: attn@v of pair p-1 and an fc chunk of the previous
    sentence are interleaved between pair p's energy/exp work so the PE
    stays dense (HAM stays at full clock).
  - V projection and fc_out weight folded on the host (wcomb); the fc bias
    is added on the host, so fc output is a pure bf16 matmul result.
"""

import math

import numpy as np

import concourse.bass as bass
import concourse.tile as tile
from concourse import bacc, mybir
from concourse import bass_utils

# problem shapes (hardcoded per the harness contract)
N, S, L, E, H = 8, 2, 512, 1024, 16
D = E // H  # 64
P = 128
NCORES = 8
LC = L // P  # 4 key chunks
NP = H // 2  # 8 head pairs
SCALE = 1.0 / math.sqrt(float(L))

F32 = mybir.dt.float32
BF16 = mybir.dt.bfloat16

# chunks whose rowsum rides the ACT accumulator (per-chunk exp); the rest use
# a batched duo exp (2x rate) with DVE tensor_scalar+accum rowsums. Tune for
# ACT/DVE balance.
ACT_CHUNKS = (0,)
WARMUP_MMS = 16  # dummy matmuls during the DMA ramp to lift HAM to 2.4 GHz


def build_kernel_body(tc, outs, ins, QP):
    nc = tc.nc

    xq, xk, xv = ins["yq"], ins["xk"], ins["xv"]
    wcomb, npads = ins["wcomb"], ins["npads"]
    outT = outs["outT"]

    import contextlib

    with contextlib.ExitStack() as ctx:
        ek = ctx.enter_context
        consts = ek(tc.tile_pool(name="consts", bufs=1))
        xkpool = ek(tc.tile_pool(name="xk", bufs=2))
        yqpool = ek(tc.tile_pool(name="yq", bufs=2))
        xvpool = ek(tc.tile_pool(name="xv", bufs=2))
        atpool = ek(tc.tile_pool(name="at", bufs=10))
        xvspool = ek(tc.tile_pool(name="xvs", bufs=1))
        ztpool = ek(tc.tile_pool(name="zt", bufs=2))
        sumpool = ek(tc.tile_pool(name="sums", bufs=4))
        outpool = ek(tc.tile_pool(name="out", bufs=3))
        pp_e = ek(tc.tile_pool(name="pp_e", bufs=2, space="PSUM"))  # 2x2 banks
        pp_z = ek(tc.tile_pool(name="pp_z", bufs=2, space="PSUM"))  # 2x1 bank
        pp_f = ek(tc.tile_pool(name="pp_f", bufs=2, space="PSUM"))  # 2x1 bank

        wcomb_sb = consts.tile([P, NP, E], BF16, tag="wcomb")
        npad_sb = consts.tile([P, S], F32, tag="npad")
        xvs_all = xvspool.tile([P, LC, E], BF16, tag="xvs")
        consts_loaded = [False]

        def load_consts():
            nc.sync.dma_start(wcomb_sb[:], wcomb[:])
            nc.sync.dma_start(npad_sb[:], npads[:])
            consts_loaded[0] = True

        # dummy dense matmuls during the DMA ramp: lift the PE HAM clock
        # gate to 8/8 (2.4 GHz) before real work arrives
        warm_in = consts.tile([P, 512], BF16, tag="warm")
        nc.vector.memset(warm_in[:], 0.0)
        wp = pp_f.tile([P, 512], F32, tag="fc", name="warmup")
        for i in range(WARMUP_MMS):
            nc.tensor.matmul(wp[:], warm_in[:, :128], warm_in[:],
                             start=True, stop=True)

        state = {}

        def dummy_mms(dst, n):
            # dense filler matmuls into an unused PSUM region: keep the PE's
            # HAM activity monitor at the 2.4 GHz clock through pipeline
            # bubbles. dst must be [P, k>=64] and otherwise unread.
            w = min(dst.shape[-1], 192)
            for _ in range(n):
                nc.tensor.matmul(dst[:, :w], warm_in[:, :128], warm_in[:, :w],
                                 start=True, stop=True)

        def emit_attnv(s, p, ZT, ats):
            zp = pp_z.tile([P, 512], F32, tag="z", name=f"zp_{s}_{p}")
            dummy_mms(zp[:, QP:512], 3)
            for c in range(LC):
                nc.tensor.matmul(
                    zp[0:64, :QP], xvs_all[:, c, (2 * p) * D:(2 * p + 1) * D],
                    ats[c][:, 0],
                    start=(c == 0), stop=(c == LC - 1), skip_group_check=True)
                nc.tensor.matmul(
                    zp[64:128, :QP], xvs_all[:, c, (2 * p + 1) * D:(2 * p + 2) * D],
                    ats[c][:, 1],
                    start=(c == 0), stop=(c == LC - 1), skip_group_check=True)
            nc.vector.tensor_copy(ZT[:, p, :], zp[:, :QP])

        def emit_fc(s, jt, ZT):
            fp = pp_f.tile([P, 512], F32, tag="fc", name=f"fp_{s}_{jt}")
            dummy_mms(fp[:, QP:512], 2)
            for eo in range(NP):
                nc.tensor.matmul(
                    fp[:, :QP], wcomb_sb[:, eo, jt * P:(jt + 1) * P],
                    ZT[:, eo, :],
                    start=(eo == 0), stop=(eo == NP - 1))
            ob = outpool.tile([P, QP], BF16, tag="ob", name=f"ob_{s}_{jt}")
            # PSUM->SBUF bf16 move on the ACT engine (DVE is rowsum-bound)
            nc.scalar.copy(ob[:], fp[:, :QP])
            nc.sync.dma_start(outT[s, jt], ob[:])

        for s in range(S):
            sb_tag = s % 2
            xk_sb = xkpool.tile([P, NP, L], BF16, tag=f"xk{sb_tag}")
            yq_sb = yqpool.tile([P, NP, QP], BF16, tag=f"yq{sb_tag}")
            nc.sync.dma_start(xk_sb[:, 0], xk[s, 0])
            nc.sync.dma_start(yq_sb[:, 0], xq[s, 0])
            xv_sb = xvpool.tile([P, LC, E], BF16, tag=f"xv{sb_tag}")
            nc.sync.dma_start(xv_sb[:], xv[s])
            for p in range(1, NP):
                nc.sync.dma_start(xk_sb[:, p], xk[s, p])
                nc.sync.dma_start(yq_sb[:, p], xq[s, p])
            if not consts_loaded[0]:
                load_consts()

            ZT = ztpool.tile([P, NP, QP], BF16, tag=f"zt{sb_tag}", name=f"zt_{s}")
            prev_ats = None
            for p in range(NP):
                # rsum[:, c, j] = sum_q at[c, j, q] (per key, f32)
                rsum = sumpool.tile([P, LC, 2], F32, tag="rsum", name=f"rs_{s}_{p}")
                ats = []
                for c in range(LC):
                    duo = pp_e.tile([P, 2, 512], F32, tag="duo", name="duo")
                    nc.tensor.matmul(
                        duo[:, 0, :QP], xk_sb[0:64, p, c * P:(c + 1) * P],
                        yq_sb[0:64, p], start=True, stop=True)
                    nc.tensor.matmul(
                        duo[:, 1, :QP], xk_sb[64:128, p, c * P:(c + 1) * P],
                        yq_sb[64:128, p], start=True, stop=True)
                    dummy_mms(duo[:, 0, QP:512], 2)
                    at = atpool.tile([P, 2, QP], BF16, tag="at",
                                     name=f"at_{s}_{p}_{c}")
                    ats.append(at)
                    if c in ACT_CHUNKS:
                        # per-chunk exp with rowsum on the ACT accumulator
                        for j in range(2):
                            nc.scalar.activation(
                                at[:, j], duo[:, j, :QP],
                                mybir.ActivationFunctionType.Exp, scale=SCALE,
                                accum_out=rsum[:, c, j:j + 1])
                    else:
                        # batched duo exp (2x); rowsums on DVE via TS+accum
                        nc.scalar.activation(
                            at[:], duo[:, :, :QP],
                            mybir.ActivationFunctionType.Exp, scale=SCALE)
                        for j in range(2):
                            nc.vector.tensor_scalar(
                                at[:, j], at[:, j], 1.0,
                                None, mybir.AluOpType.mult,
                                op1=mybir.AluOpType.add,
                                accum_out=rsum[:, c, j:j + 1])
                # den = rsum - npad (pad cols contribute exp(0)=1), recip
                nc.vector.tensor_scalar(
                    rsum[:, :, :], rsum[:, :, :], npad_sb[:, s:s + 1], None,
                    mybir.AluOpType.subtract)
                recipt = sumpool.tile([P, LC, 2], F32, tag="recip")
                nc.vector.reciprocal(recipt[:], rsum[:])
                # xvs[:, c, hA/hB] = xv * (1/den), one fused op per pair
                nc.vector.scalar_tensor_tensor(
                    xvs_all[:, :, (2 * p) * D:(2 * p + 2) * D].rearrange(
                        "p c (j d) -> p c j d", d=D),
                    xv_sb[:, :, (2 * p) * D:(2 * p + 2) * D].rearrange(
                        "p c (j d) -> p c j d", d=D),
                    1.0,
                    recipt[:, :, :, None].to_broadcast((P, LC, 2, D)),
                    mybir.AluOpType.mult, mybir.AluOpType.mult)
                # software pipeline: attn@v of the previous pair + one fc
                # chunk of the previous sentence keep the PE dense
                if p > 0:
                    emit_attnv(s, p - 1, ZT, prev_ats)
                if s > 0:
                    emit_fc(s - 1, p, state["ZT_prev"])
                prev_ats = ats
            emit_attnv(s, NP - 1, ZT, prev_ats)
            state["ZT_prev"] = ZT

        for jt in range(NP):
            emit_fc(S - 1, jt, state["ZT_prev"])


def host_prepare(values, keys, query, mask, Wv, Wk, Wq, Wo, bo):
    """Host-side sharding + layout + query compaction + weight folding."""
    import ml_dtypes
    bf = ml_dtypes.bfloat16

    values = np.asarray(values, dtype=np.float32)
    keys = np.asarray(keys, dtype=np.float32)
    query = np.asarray(query, dtype=np.float32)
    mask = np.asarray(mask)
    Wv = np.asarray(Wv, dtype=np.float32)
    Wk = np.asarray(Wk, dtype=np.float32)
    Wq = np.asarray(Wq, dtype=np.float32)
    Wo = np.asarray(Wo, dtype=np.float32)
    bo_np = np.ascontiguousarray(np.asarray(bo, dtype=np.float32))

    keep = mask[:, :, :, 0] != 0  # (N, S, L) True = query position survives
    cnt = keep.sum(-1)  # (N, S)
    QP = int(np.ceil(max(int(cnt.max()), 1) / 64) * 64)
    QP = min(max(QP, 64), L)
    order = np.argsort(~keep, axis=-1, kind="stable")  # (N, S, L)
    gidx = order[:, :, :QP]  # (N, S, QP)
    pad = np.arange(QP)[None, None, :] >= cnt[:, :, None]  # (N, S, QP)

    qT = query.transpose(0, 1, 3, 2).reshape(N, S, H, D, L)
    qTc = np.take_along_axis(
        qT, gidx[:, :, None, None, :].repeat(H, 2).repeat(D, 3), axis=4)
    qTc[pad[:, :, None, None, :].repeat(H, 2).repeat(D, 3)] = 0.0
    M = Wk.T @ Wq  # (D, D): energy^T = k^T M q
    yq = np.moveaxis(np.tensordot(M, qTc, axes=([1], [3])), 0, 3)
    yq = np.ascontiguousarray(yq.reshape(N, S, NP, P, QP).astype(bf))

    kT = keys.transpose(0, 1, 3, 2).reshape(N, S, H, D, L)
    xk = np.ascontiguousarray(kT.reshape(N, S, NP, P, L).astype(bf))

    xv = np.ascontiguousarray(
        values.reshape(N, S, LC, P, E).transpose(0, 1, 3, 2, 4).astype(bf))

    wcomb = np.zeros((E, E), np.float32)
    for h in range(H):
        wcomb[h * D:(h + 1) * D, :] = Wv.T @ Wo[:, h * D:(h + 1) * D].T
    wcomb_r = np.ascontiguousarray(
        wcomb.reshape(NP, P, E).transpose(1, 0, 2).astype(bf))

    npad_f = (QP - cnt).astype(np.float32)  # (N, S)
    npads = np.ascontiguousarray(
        np.broadcast_to(npad_f[:, None, :], (N, P, S)).copy())

    in_maps = []
    for n in range(NCORES):
        in_maps.append({
            "yq": yq[n], "xk": xk[n], "xv": xv[n],
            "wcomb": wcomb_r, "npads": npads[n],
        })
    return in_maps, QP, order, cnt, bo_np


_NC_CACHE = {}


def _get_program(QP):
    nc = _NC_CACHE.get(QP)
    if nc is not None:
        return nc
    nc = bacc.Bacc("TRN2", target_bir_lowering=False, debug=False,
                   num_devices=NCORES)
    ins = {
        "yq": nc.dram_tensor("yq", (S, NP, P, QP), BF16, kind="ExternalInput").ap(),
        "xk": nc.dram_tensor("xk", (S, NP, P, L), BF16, kind="ExternalInput").ap(),
        "xv": nc.dram_tensor("xv", (S, P, LC, E), BF16, kind="ExternalInput").ap(),
        "wcomb": nc.dram_tensor("wcomb", (P, NP, E), BF16, kind="ExternalInput").ap(),
        "npads": nc.dram_tensor("npads", (P, S), F32, kind="ExternalInput").ap(),
    }
    outs = {
        "outT": nc.dram_tensor("outT", (S, E // P, P, QP), BF16,
                               kind="ExternalOutput").ap(),
    }
    with tile.TileContext(nc) as tc:
        build_kernel_body(tc, outs, ins, QP)
    nc.compile()
    _NC_CACHE[QP] = nc
    return nc


def run(inputs: dict, trace: bool = False):
    """Run on 8 cores; returns (full_output, BassKernelResults)."""
    in_maps, QP, order, cnt, bo_np = host_prepare(**inputs)
    nc = _get_program(QP)
    res = bass_utils.run_bass_kernel_spmd(
        nc, in_maps, core_ids=list(range(NCORES)), trace=trace,
    )
    out = np.empty((N, S, L, E), np.float32)
    out[:] = bo_np  # masked query rows: attention output is 0, fc adds bo
    for n in range(NCORES):
        oT = np.asarray(res.results[n]["outT"], dtype=np.float32)  # (S,8,P,QP)
        for s in range(S):
            c = int(cnt[n, s])
            if c:
                out[n, s, order[n, s, :c], :] = (
                    oT[s].reshape(E, QP)[:, :c].T + bo_np)
    return out, res


def kernel(**inputs) -> np.ndarray:
    out, _ = run(inputs, trace=False)
    return out


# revision 19
# speedup vs baseline: 1.0371x; 1.0371x over previous
"""Trainium2 Bass kernel for nn_EntailmentSelfAttention (8-core data parallel).

Mapping (one n per NeuronCore; S=2 sentences iterated inside):
  - Transposed on-chip layout: head-dim on partitions, sequence on the free
    axis, so the softmax (over queries) reduces along the free axis.
  - q-side projection folded on the HOST: yq = (Wk^T Wq) @ q^T; the on-chip
    energy matmul is raw keys (stationary) x yq (moving), 64-deep contraction.
  - Heads processed in PAIRS packed on SBUF partitions (head A on 0:64, head
    B on 64:128). The two 64-row energy matmuls use tile_position row groups
    (0,0)/(64,0) and run CONCURRENTLY in the PE array; the two attn@v matmuls
    are column-group paired (0,0)/(0,64) into one PSUM bank.
  - Masked query positions dropped on the host (compaction to QP columns).
    Pad columns have yq = 0 -> energy 0 -> exp = 1; the pad count is
    subtracted from the softmax denominators on-chip (npads input).
  - exp on the scalar engine: for DVE_CHUNKS, one ACTIVATE covers both
    chunks of a duo via a 2-bank strided PSUM AP (bf16 out runs 2x) and the
    rowsums come from DVE tensor_scalar+accum_out (in-place); for the other
    chunks, per-chunk ACTIVATE with accum_out so the rowsum rides the ACT
    engine (load balance across ACT/DVE).
  - xv scaling by 1/rowsum: one fused scalar_tensor_tensor per pair with a
    broadcast reciprocal.
  - Software pipeline: attn@v of pair p-1 and an fc chunk of the previous
    sentence are interleaved between pair p's energy/exp work so the PE
    stays dense (HAM stays at full clock).
  - V projection and fc_out weight folded on the host (wcomb); the fc bias
    is added on the host, so fc output is a pure bf16 matmul result.
"""

import math

import numpy as np

import concourse.bass as bass
import concourse.tile as tile
from concourse import bacc, mybir
from concourse import bass_utils

# problem shapes (hardcoded per the harness contract)
N, S, L, E, H = 8, 2, 512, 1024, 16
D = E // H  # 64
P = 128
NCORES = 8
LC = L // P  # 4 key chunks
NP = H // 2  # 8 head pairs
SCALE = 1.0 / math.sqrt(float(L))

F32 = mybir.dt.float32
BF16 = mybir.dt.bfloat16

# chunks whose rowsum rides the ACT accumulator (per-chunk exp); the rest use
# a batched duo exp (2x rate) with DVE tensor_scalar+accum rowsums. Tune for
# ACT/DVE balance.
ACT_CHUNKS = (0,)
WARMUP_MMS = 16  # dummy matmuls during the DMA ramp to lift HAM to 2.4 GHz


def build_kernel_body(tc, outs, ins, QP):
    nc = tc.nc

    xq, xk, xv = ins["yq"], ins["xk"], ins["xv"]
    wcomb, npads = ins["wcomb"], ins["npads"]
    outT = outs["outT"]

    import contextlib

    with contextlib.ExitStack() as ctx:
        ek = ctx.enter_context
        consts = ek(tc.tile_pool(name="consts", bufs=1))
        xkpool = ek(tc.tile_pool(name="xk", bufs=2))
        yqpool = ek(tc.tile_pool(name="yq", bufs=2))
        xvpool = ek(tc.tile_pool(name="xv", bufs=2))
        atpool = ek(tc.tile_pool(name="at", bufs=10))
        xvspool = ek(tc.tile_pool(name="xvs", bufs=1))
        ztpool = ek(tc.tile_pool(name="zt", bufs=2))
        sumpool = ek(tc.tile_pool(name="sums", bufs=4))
        outpool = ek(tc.tile_pool(name="out", bufs=3))
        pp_e = ek(tc.tile_pool(name="pp_e", bufs=2, space="PSUM"))  # 2x2 banks
        pp_z = ek(tc.tile_pool(name="pp_z", bufs=2, space="PSUM"))  # 2x1 bank
        pp_f = ek(tc.tile_pool(name="pp_f", bufs=2, space="PSUM"))  # 2x1 bank

        wcomb_sb = consts.tile([P, NP, E], BF16, tag="wcomb")
        npad_sb = consts.tile([P, S], F32, tag="npad")
        xvs_all = xvspool.tile([P, LC, E], BF16, tag="xvs")
        consts_loaded = [False]

        def load_consts():
            nc.sync.dma_start(wcomb_sb[:], wcomb[:])
            nc.sync.dma_start(npad_sb[:], npads[:])
            consts_loaded[0] = True

        # dummy dense matmuls during the DMA ramp: lift the PE HAM clock
        # gate to 8/8 (2.4 GHz) before real work arrives
        warm_in = consts.tile([P, 512], BF16, tag="warm")
        nc.vector.memset(warm_in[:], 0.0)
        wp = pp_f.tile([P, 512], F32, tag="fc", name="warmup")
        for i in range(WARMUP_MMS):
            nc.tensor.matmul(wp[:], warm_in[:, :128], warm_in[:],
                             start=True, stop=True)

        state = {}

        def dummy_mms(dst, n):
            # dense filler matmuls into an unused PSUM region: keep the PE's
            # HAM activity monitor at the 2.4 GHz clock through pipeline
            # bubbles. dst must be [P, k>=64] and otherwise unread.
            w = min(dst.shape[-1], 192)
            for _ in range(n):
                nc.tensor.matmul(dst[:, :w], warm_in[:, :128], warm_in[:, :w],
                                 start=True, stop=True)

        def emit_attnv(s, p, ZT, ats):
            zp = pp_z.tile([P, 512], F32, tag="z", name=f"zp_{s}_{p}")
            for c in range(LC):
                nc.tensor.matmul(
                    zp[0:64, :QP], xvs_all[:, c, (2 * p) * D:(2 * p + 1) * D],
                    ats[c][:, 0],
                    start=(c == 0), stop=(c == LC - 1), skip_group_check=True)
                nc.tensor.matmul(
                    zp[64:128, :QP], xvs_all[:, c, (2 * p + 1) * D:(2 * p + 2) * D],
                    ats[c][:, 1],
                    start=(c == 0), stop=(c == LC - 1), skip_group_check=True)
            nc.vector.tensor_copy(ZT[:, p, :], zp[:, :QP])

        def emit_fc(s, jt, ZT):
            fp = pp_f.tile([P, 512], F32, tag="fc", name=f"fp_{s}_{jt}")
            for eo in range(NP):
                nc.tensor.matmul(
                    fp[:, :QP], wcomb_sb[:, eo, jt * P:(jt + 1) * P],
                    ZT[:, eo, :],
                    start=(eo == 0), stop=(eo == NP - 1))
            ob = outpool.tile([P, QP], BF16, tag="ob", name=f"ob_{s}_{jt}")
            # PSUM->SBUF bf16 move on the ACT engine (DVE is rowsum-bound)
            nc.scalar.copy(ob[:], fp[:, :QP])
            nc.sync.dma_start(outT[s, jt], ob[:])

        for s in range(S):
            sb_tag = s % 2
            xk_sb = xkpool.tile([P, NP, L], BF16, tag=f"xk{sb_tag}")
            yq_sb = yqpool.tile([P, NP, QP], BF16, tag=f"yq{sb_tag}")
            nc.sync.dma_start(xk_sb[:, 0], xk[s, 0])
            nc.sync.dma_start(yq_sb[:, 0], xq[s, 0])
            xv_sb = xvpool.tile([P, LC, E], BF16, tag=f"xv{sb_tag}")
            nc.sync.dma_start(xv_sb[:], xv[s])
            for p in range(1, NP):
                nc.sync.dma_start(xk_sb[:, p], xk[s, p])
                nc.sync.dma_start(yq_sb[:, p], xq[s, p])
            if not consts_loaded[0]:
                load_consts()

            ZT = ztpool.tile([P, NP, QP], BF16, tag=f"zt{sb_tag}", name=f"zt_{s}")
            prev_ats = None
            for p in range(NP):
                # rsum[:, c, j] = sum_q at[c, j, q] (per key, f32)
                rsum = sumpool.tile([P, LC, 2], F32, tag="rsum", name=f"rs_{s}_{p}")
                ats = []
                for c in range(LC):
                    duo = pp_e.tile([P, 2, 512], F32, tag="duo", name="duo")
                    nc.tensor.matmul(
                        duo[:, 0, :QP], xk_sb[0:64, p, c * P:(c + 1) * P],
                        yq_sb[0:64, p], start=True, stop=True)
                    nc.tensor.matmul(
                        duo[:, 1, :QP], xk_sb[64:128, p, c * P:(c + 1) * P],
                        yq_sb[64:128, p], start=True, stop=True)
                    dummy_mms(duo[:, 0, QP:512], 1)
                    at = atpool.tile([P, 2, QP], BF16, tag="at",
                                     name=f"at_{s}_{p}_{c}")
                    ats.append(at)
                    if c in ACT_CHUNKS:
                        # per-chunk exp with rowsum on the ACT accumulator
                        for j in range(2):
                            nc.scalar.activation(
                                at[:, j], duo[:, j, :QP],
                                mybir.ActivationFunctionType.Exp, scale=SCALE,
                                accum_out=rsum[:, c, j:j + 1])
                    else:
                        # batched duo exp (2x); rowsums on DVE via TS+accum
                        nc.scalar.activation(
                            at[:], duo[:, :, :QP],
                            mybir.ActivationFunctionType.Exp, scale=SCALE)
                        nc.vector.tensor_reduce(
                            rsum[:, c, :], at[:],
                            axis=mybir.AxisListType.X, op=mybir.AluOpType.add)
                # den = rsum - npad (pad cols contribute exp(0)=1), recip
                nc.vector.tensor_scalar(
                    rsum[:, :, :], rsum[:, :, :], npad_sb[:, s:s + 1], None,
                    mybir.AluOpType.subtract)
                recipt = sumpool.tile([P, LC, 2], F32, tag="recip")
                nc.vector.reciprocal(recipt[:], rsum[:])
                # xvs[:, c, hA/hB] = xv * (1/den), one fused op per pair
                nc.vector.scalar_tensor_tensor(
                    xvs_all[:, :, (2 * p) * D:(2 * p + 2) * D].rearrange(
                        "p c (j d) -> p c j d", d=D),
                    xv_sb[:, :, (2 * p) * D:(2 * p + 2) * D].rearrange(
                        "p c (j d) -> p c j d", d=D),
                    1.0,
                    recipt[:, :, :, None].to_broadcast((P, LC, 2, D)),
                    mybir.AluOpType.mult, mybir.AluOpType.mult)
                # software pipeline: attn@v of the previous pair + one fc
                # chunk of the previous sentence keep the PE dense
                if p > 0:
                    emit_attnv(s, p - 1, ZT, prev_ats)
                if s > 0:
                    emit_fc(s - 1, p, state["ZT_prev"])
                prev_ats = ats
            emit_attnv(s, NP - 1, ZT, prev_ats)
            state["ZT_prev"] = ZT

        for jt in range(NP):
            emit_fc(S - 1, jt, state["ZT_prev"])


def host_prepare(values, keys, query, mask, Wv, Wk, Wq, Wo, bo):
    """Host-side sharding + layout + query compaction + weight folding."""
    import ml_dtypes
    bf = ml_dtypes.bfloat16

    values = np.asarray(values, dtype=np.float32)
    keys = np.asarray(keys, dtype=np.float32)
    query = np.asarray(query, dtype=np.float32)
    mask = np.asarray(mask)
    Wv = np.asarray(Wv, dtype=np.float32)
    Wk = np.asarray(Wk, dtype=np.float32)
    Wq = np.asarray(Wq, dtype=np.float32)
    Wo = np.asarray(Wo, dtype=np.float32)
    bo_np = np.ascontiguousarray(np.asarray(bo, dtype=np.float32))

    keep = mask[:, :, :, 0] != 0  # (N, S, L) True = query position survives
    cnt = keep.sum(-1)  # (N, S)
    QP = int(np.ceil(max(int(cnt.max()), 1) / 64) * 64)
    QP = min(max(QP, 64), L)
    order = np.argsort(~keep, axis=-1, kind="stable")  # (N, S, L)
    gidx = order[:, :, :QP]  # (N, S, QP)
    pad = np.arange(QP)[None, None, :] >= cnt[:, :, None]  # (N, S, QP)

    qT = query.transpose(0, 1, 3, 2).reshape(N, S, H, D, L)
    qTc = np.take_along_axis(
        qT, gidx[:, :, None, None, :].repeat(H, 2).repeat(D, 3), axis=4)
    qTc[pad[:, :, None, None, :].repeat(H, 2).repeat(D, 3)] = 0.0
    M = Wk.T @ Wq  # (D, D): energy^T = k^T M q
    yq = np.moveaxis(np.tensordot(M, qTc, axes=([1], [3])), 0, 3)
    yq = np.ascontiguousarray(yq.reshape(N, S, NP, P, QP).astype(bf))

    kT = keys.transpose(0, 1, 3, 2).reshape(N, S, H, D, L)
    xk = np.ascontiguousarray(kT.reshape(N, S, NP, P, L).astype(bf))

    xv = np.ascontiguousarray(
        values.reshape(N, S, LC, P, E).transpose(0, 1, 3, 2, 4).astype(bf))

    wcomb = np.zeros((E, E), np.float32)
    for h in range(H):
        wcomb[h * D:(h + 1) * D, :] = Wv.T @ Wo[:, h * D:(h + 1) * D].T
    wcomb_r = np.ascontiguousarray(
        wcomb.reshape(NP, P, E).transpose(1, 0, 2).astype(bf))

    npad_f = (QP - cnt).astype(np.float32)  # (N, S)
    npads = np.ascontiguousarray(
        np.broadcast_to(npad_f[:, None, :], (N, P, S)).copy())

    in_maps = []
    for n in range(NCORES):
        in_maps.append({
            "yq": yq[n], "xk": xk[n], "xv": xv[n],
            "wcomb": wcomb_r, "npads": npads[n],
        })
    return in_maps, QP, order, cnt, bo_np


_NC_CACHE = {}


def _get_program(QP):
    nc = _NC_CACHE.get(QP)
    if nc is not None:
        return nc
    nc = bacc.Bacc("TRN2", target_bir_lowering=False, debug=False,
                   num_devices=NCORES)
    ins = {
        "yq": nc.dram_tensor("yq", (S, NP, P, QP), BF16, kind="ExternalInput").ap(),
        "xk": nc.dram_tensor("xk", (S, NP, P, L), BF16, kind="ExternalInput").ap(),
        "xv": nc.dram_tensor("xv", (S, P, LC, E), BF16, kind="ExternalInput").ap(),
        "wcomb": nc.dram_tensor("wcomb", (P, NP, E), BF16, kind="ExternalInput").ap(),
        "npads": nc.dram_tensor("npads", (P, S), F32, kind="ExternalInput").ap(),
    }
    outs = {
        "outT": nc.dram_tensor("outT", (S, E // P, P, QP), BF16,
                               kind="ExternalOutput").ap(),
    }
    with tile.TileContext(nc) as tc:
        build_kernel_body(tc, outs, ins, QP)
    nc.compile()
    _NC_CACHE[QP] = nc
    return nc


def run(inputs: dict, trace: bool = False):
    """Run on 8 cores; returns (full_output, BassKernelResults)."""
    in_maps, QP, order, cnt, bo_np = host_prepare(**inputs)
    nc = _get_program(QP)
    res = bass_utils.run_bass_kernel_spmd(
        nc, in_maps, core_ids=list(range(NCORES)), trace=trace,
    )
    out = np.empty((N, S, L, E), np.float32)
    out[:] = bo_np  # masked query rows: attention output is 0, fc adds bo
    for n in range(NCORES):
        oT = np.asarray(res.results[n]["outT"], dtype=np.float32)  # (S,8,P,QP)
        for s in range(S):
            c = int(cnt[n, s])
            if c:
                out[n, s, order[n, s, :c], :] = (
                    oT[s].reshape(E, QP)[:, :c].T + bo_np)
    return out, res


def kernel(**inputs) -> np.ndarray:
    out, _ = run(inputs, trace=False)
    return out


# revision 22
# speedup vs baseline: 1.0749x; 1.0365x over previous
"""Trainium2 Bass kernel for nn_EntailmentSelfAttention (8-core data parallel).

Mapping (one n per NeuronCore; S=2 sentences iterated inside):
  - Transposed on-chip layout: head-dim on partitions, sequence on the free
    axis, so the softmax (over queries) reduces along the free axis.
  - q-side projection folded on the HOST: yq = (Wk^T Wq) @ q^T; the on-chip
    energy matmul is raw keys (stationary) x yq (moving), 64-deep contraction.
  - Heads processed in PAIRS packed on SBUF partitions (head A on 0:64, head
    B on 64:128). The two 64-row energy matmuls use tile_position row groups
    (0,0)/(64,0) and run CONCURRENTLY in the PE array; the two attn@v matmuls
    are column-group paired (0,0)/(0,64) into one PSUM bank.
  - Masked query positions dropped on the host (compaction to QP columns).
    Pad columns have yq = 0 -> energy 0 -> exp = 1; the pad count is
    subtracted from the softmax denominators on-chip (npads input).
  - exp on the scalar engine: for DVE_CHUNKS, one ACTIVATE covers both
    chunks of a duo via a 2-bank strided PSUM AP (bf16 out runs 2x) and the
    rowsums come from DVE tensor_scalar+accum_out (in-place); for the other
    chunks, per-chunk ACTIVATE with accum_out so the rowsum rides the ACT
    engine (load balance across ACT/DVE).
  - xv scaling by 1/rowsum: one fused scalar_tensor_tensor per pair with a
    broadcast reciprocal.
  - Software pipeline: attn@v of pair p-1 and an fc chunk of the previous
    sentence are interleaved between pair p's energy/exp work so the PE
    stays dense (HAM stays at full clock).
  - V projection and fc_out weight folded on the host (wcomb); the fc bias
    is added on the host, so fc output is a pure bf16 matmul result.
"""

import math

import numpy as np

import concourse.bass as bass
import concourse.tile as tile
from concourse import bacc, mybir
from concourse import bass_utils

# problem shapes (hardcoded per the harness contract)
N, S, L, E, H = 8, 2, 512, 1024, 16
D = E // H  # 64
P = 128
NCORES = 8
LC = L // P  # 4 key chunks
NP = H // 2  # 8 head pairs
SCALE = 1.0 / math.sqrt(float(L))

F32 = mybir.dt.float32
BF16 = mybir.dt.bfloat16

# chunks whose rowsum rides the ACT accumulator (per-chunk exp); the rest use
# a batched duo exp (2x rate) with DVE tensor_scalar+accum rowsums. Tune for
# ACT/DVE balance.
ACT_CHUNKS = (0,)
WARMUP_MMS = 16  # dummy matmuls during the DMA ramp to lift HAM to 2.4 GHz


def build_kernel_body(tc, outs, ins, QP):
    nc = tc.nc

    xq, xk, xv = ins["yq"], ins["xk"], ins["xv"]
    wcomb, npads = ins["wcomb"], ins["npads"]
    outT = outs["outT"]

    import contextlib

    with contextlib.ExitStack() as ctx:
        ek = ctx.enter_context
        consts = ek(tc.tile_pool(name="consts", bufs=1))
        xkpool = ek(tc.tile_pool(name="xk", bufs=2))
        yqpool = ek(tc.tile_pool(name="yq", bufs=2))
        xvpool = ek(tc.tile_pool(name="xv", bufs=2))
        atpool = ek(tc.tile_pool(name="at", bufs=10))
        xvspool = ek(tc.tile_pool(name="xvs", bufs=1))
        ztpool = ek(tc.tile_pool(name="zt", bufs=2))
        sumpool = ek(tc.tile_pool(name="sums", bufs=4))
        outpool = ek(tc.tile_pool(name="out", bufs=3))
        pp_e = ek(tc.tile_pool(name="pp_e", bufs=2, space="PSUM"))  # 2x2 banks
        pp_z = ek(tc.tile_pool(name="pp_z", bufs=2, space="PSUM"))  # 2x1 bank
        pp_f = ek(tc.tile_pool(name="pp_f", bufs=2, space="PSUM"))  # 2x1 bank

        wcomb_sb = consts.tile([P, NP, E], BF16, tag="wcomb")
        npad_sb = consts.tile([P, S], F32, tag="npad")
        xvs_all = xvspool.tile([P, LC, E], BF16, tag="xvs")
        consts_loaded = [False]

        def load_consts():
            nc.sync.dma_start(wcomb_sb[:], wcomb[:])
            nc.sync.dma_start(npad_sb[:], npads[:])
            consts_loaded[0] = True

        # dummy dense matmuls during the DMA ramp: lift the PE HAM clock
        # gate to 8/8 (2.4 GHz) before real work arrives
        warm_in = consts.tile([P, 512], BF16, tag="warm")
        nc.vector.memset(warm_in[:], 0.0)
        wp = pp_f.tile([P, 512], F32, tag="fc", name="warmup")
        for i in range(WARMUP_MMS):
            nc.tensor.matmul(wp[:], warm_in[:, :128], warm_in[:],
                             start=True, stop=True)
        # trigger the exp ACT_TABLE_LOAD (~2.7us) during the DMA ramp
        warm_exp = consts.tile([P, 1], BF16, tag="warm_exp")
        nc.scalar.activation(warm_exp[:], warm_in[:, :1],
                             mybir.ActivationFunctionType.Exp, scale=1.0)

        state = {}

        def dummy_mms(dst, n):
            # dense filler matmuls into an unused PSUM region: keep the PE's
            # HAM activity monitor at the 2.4 GHz clock through pipeline
            # bubbles. dst must be [P, k>=64] and otherwise unread.
            w = min(dst.shape[-1], 192)
            for _ in range(n):
                nc.tensor.matmul(dst[:, :w], warm_in[:, :128], warm_in[:, :w],
                                 start=True, stop=True)

        def emit_attnv(s, p, ZT, ats):
            zp = pp_z.tile([P, 512], F32, tag="z", name=f"zp_{s}_{p}")
            for c in range(LC):
                nc.tensor.matmul(
                    zp[0:64, :QP], xvs_all[:, c, (2 * p) * D:(2 * p + 1) * D],
                    ats[c][:, 0],
                    start=(c == 0), stop=(c == LC - 1), skip_group_check=True)
                nc.tensor.matmul(
                    zp[64:128, :QP], xvs_all[:, c, (2 * p + 1) * D:(2 * p + 2) * D],
                    ats[c][:, 1],
                    start=(c == 0), stop=(c == LC - 1), skip_group_check=True)
            nc.vector.tensor_copy(ZT[:, p, :], zp[:, :QP])

        def emit_fc(s, jt, ZT):
            fp = pp_f.tile([P, 512], F32, tag="fc", name=f"fp_{s}_{jt}")
            for eo in range(NP):
                nc.tensor.matmul(
                    fp[:, :QP], wcomb_sb[:, eo, jt * P:(jt + 1) * P],
                    ZT[:, eo, :],
                    start=(eo == 0), stop=(eo == NP - 1))
            ob = outpool.tile([P, QP], BF16, tag="ob", name=f"ob_{s}_{jt}")
            # PSUM->SBUF bf16 move, alternating engines for load balance
            if jt % 2 == 0:
                nc.scalar.copy(ob[:], fp[:, :QP])
            else:
                nc.vector.tensor_copy(ob[:], fp[:, :QP])
            nc.sync.dma_start(outT[s, jt], ob[:])

        for s in range(S):
            sb_tag = s % 2
            xk_sb = xkpool.tile([P, NP, L], BF16, tag=f"xk{sb_tag}")
            yq_sb = yqpool.tile([P, NP, QP], BF16, tag=f"yq{sb_tag}")
            nc.sync.dma_start(xk_sb[:, 0], xk[s, 0])
            nc.sync.dma_start(yq_sb[:, 0], xq[s, 0])
            xv_sb = xvpool.tile([P, LC, E], BF16, tag=f"xv{sb_tag}")
            nc.sync.dma_start(xv_sb[:], xv[s])
            for p in range(1, NP):
                nc.sync.dma_start(xk_sb[:, p], xk[s, p])
                nc.sync.dma_start(yq_sb[:, p], xq[s, p])
            if not consts_loaded[0]:
                load_consts()

            ZT = ztpool.tile([P, NP, QP], BF16, tag=f"zt{sb_tag}", name=f"zt_{s}")
            prev_ats = None
            for p in range(NP):
                # rsum[:, c, j] = sum_q at[c, j, q] (per key, f32)
                rsum = sumpool.tile([P, LC, 2], F32, tag="rsum", name=f"rs_{s}_{p}")
                ats = []
                for c in range(LC):
                    duo = pp_e.tile([P, 2, 512], F32, tag="duo", name="duo")
                    nc.tensor.matmul(
                        duo[:, 0, :QP], xk_sb[0:64, p, c * P:(c + 1) * P],
                        yq_sb[0:64, p], start=True, stop=True)
                    nc.tensor.matmul(
                        duo[:, 1, :QP], xk_sb[64:128, p, c * P:(c + 1) * P],
                        yq_sb[64:128, p], start=True, stop=True)
                    # sentence 0 lacks the fc interleave: extra PE filler to
                    # keep the HAM clock gate open
                    dummy_mms(duo[:, 0, QP:512], 3 if s == 0 else 1)
                    at = atpool.tile([P, 2, QP], BF16, tag="at",
                                     name=f"at_{s}_{p}_{c}")
                    ats.append(at)
                    if c in ACT_CHUNKS:
                        # per-chunk exp with rowsum on the ACT accumulator
                        for j in range(2):
                            nc.scalar.activation(
                                at[:, j], duo[:, j, :QP],
                                mybir.ActivationFunctionType.Exp, scale=SCALE,
                                accum_out=rsum[:, c, j:j + 1])
                    else:
                        # batched duo exp (2x); rowsums on DVE via TS+accum
                        nc.scalar.activation(
                            at[:], duo[:, :, :QP],
                            mybir.ActivationFunctionType.Exp, scale=SCALE)
                        nc.vector.tensor_reduce(
                            rsum[:, c, :], at[:],
                            axis=mybir.AxisListType.X, op=mybir.AluOpType.add)
                # den = rsum - npad (pad cols contribute exp(0)=1), recip
                nc.vector.tensor_scalar(
                    rsum[:, :, :], rsum[:, :, :], npad_sb[:, s:s + 1], None,
                    mybir.AluOpType.subtract)
                recipt = sumpool.tile([P, LC, 2], F32, tag="recip")
                nc.vector.reciprocal(recipt[:], rsum[:])
                # xvs[:, c, hA/hB] = xv * (1/den), one fused op per pair
                nc.vector.scalar_tensor_tensor(
                    xvs_all[:, :, (2 * p) * D:(2 * p + 2) * D].rearrange(
                        "p c (j d) -> p c j d", d=D),
                    xv_sb[:, :, (2 * p) * D:(2 * p + 2) * D].rearrange(
                        "p c (j d) -> p c j d", d=D),
                    1.0,
                    recipt[:, :, :, None].to_broadcast((P, LC, 2, D)),
                    mybir.AluOpType.mult, mybir.AluOpType.mult)
                # software pipeline: attn@v of the previous pair + one fc
                # chunk of the previous sentence keep the PE dense
                if p > 0:
                    emit_attnv(s, p - 1, ZT, prev_ats)
                if s > 0:
                    emit_fc(s - 1, p, state["ZT_prev"])
                prev_ats = ats
            emit_attnv(s, NP - 1, ZT, prev_ats)
            state["ZT_prev"] = ZT

        for jt in range(NP):
            emit_fc(S - 1, jt, state["ZT_prev"])


def host_prepare(values, keys, query, mask, Wv, Wk, Wq, Wo, bo):
    """Host-side sharding + layout + query compaction + weight folding."""
    import ml_dtypes
    bf = ml_dtypes.bfloat16

    values = np.asarray(values, dtype=np.float32)
    keys = np.asarray(keys, dtype=np.float32)
    query = np.asarray(query, dtype=np.float32)
    mask = np.asarray(mask)
    Wv = np.asarray(Wv, dtype=np.float32)
    Wk = np.asarray(Wk, dtype=np.float32)
    Wq = np.asarray(Wq, dtype=np.float32)
    Wo = np.asarray(Wo, dtype=np.float32)
    bo_np = np.ascontiguousarray(np.asarray(bo, dtype=np.float32))

    keep = mask[:, :, :, 0] != 0  # (N, S, L) True = query position survives
    cnt = keep.sum(-1)  # (N, S)
    QP = int(np.ceil(max(int(cnt.max()), 1) / 64) * 64)
    QP = min(max(QP, 64), L)
    order = np.argsort(~keep, axis=-1, kind="stable")  # (N, S, L)
    gidx = order[:, :, :QP]  # (N, S, QP)
    pad = np.arange(QP)[None, None, :] >= cnt[:, :, None]  # (N, S, QP)

    qT = query.transpose(0, 1, 3, 2).reshape(N, S, H, D, L)
    qTc = np.take_along_axis(
        qT, gidx[:, :, None, None, :].repeat(H, 2).repeat(D, 3), axis=4)
    qTc[pad[:, :, None, None, :].repeat(H, 2).repeat(D, 3)] = 0.0
    M = Wk.T @ Wq  # (D, D): energy^T = k^T M q
    yq = np.moveaxis(np.tensordot(M, qTc, axes=([1], [3])), 0, 3)
    yq = np.ascontiguousarray(yq.reshape(N, S, NP, P, QP).astype(bf))

    kT = keys.transpose(0, 1, 3, 2).reshape(N, S, H, D, L)
    xk = np.ascontiguousarray(kT.reshape(N, S, NP, P, L).astype(bf))

    xv = np.ascontiguousarray(
        values.reshape(N, S, LC, P, E).transpose(0, 1, 3, 2, 4).astype(bf))

    wcomb = np.zeros((E, E), np.float32)
    for h in range(H):
        wcomb[h * D:(h + 1) * D, :] = Wv.T @ Wo[:, h * D:(h + 1) * D].T
    wcomb_r = np.ascontiguousarray(
        wcomb.reshape(NP, P, E).transpose(1, 0, 2).astype(bf))

    npad_f = (QP - cnt).astype(np.float32)  # (N, S)
    npads = np.ascontiguousarray(
        np.broadcast_to(npad_f[:, None, :], (N, P, S)).copy())

    in_maps = []
    for n in range(NCORES):
        in_maps.append({
            "yq": yq[n], "xk": xk[n], "xv": xv[n],
            "wcomb": wcomb_r, "npads": npads[n],
        })
    return in_maps, QP, order, cnt, bo_np


_NC_CACHE = {}


def _get_program(QP):
    nc = _NC_CACHE.get(QP)
    if nc is not None:
        return nc
    nc = bacc.Bacc("TRN2", target_bir_lowering=False, debug=False,
                   num_devices=NCORES)
    ins = {
        "yq": nc.dram_tensor("yq", (S, NP, P, QP), BF16, kind="ExternalInput").ap(),
        "xk": nc.dram_tensor("xk", (S, NP, P, L), BF16, kind="ExternalInput").ap(),
        "xv": nc.dram_tensor("xv", (S, P, LC, E), BF16, kind="ExternalInput").ap(),
        "wcomb": nc.dram_tensor("wcomb", (P, NP, E), BF16, kind="ExternalInput").ap(),
        "npads": nc.dram_tensor("npads", (P, S), F32, kind="ExternalInput").ap(),
    }
    outs = {
        "outT": nc.dram_tensor("outT", (S, E // P, P, QP), BF16,
                               kind="ExternalOutput").ap(),
    }
    with tile.TileContext(nc) as tc:
        build_kernel_body(tc, outs, ins, QP)
    nc.compile()
    _NC_CACHE[QP] = nc
    return nc


def run(inputs: dict, trace: bool = False):
    """Run on 8 cores; returns (full_output, BassKernelResults)."""
    in_maps, QP, order, cnt, bo_np = host_prepare(**inputs)
    nc = _get_program(QP)
    res = bass_utils.run_bass_kernel_spmd(
        nc, in_maps, core_ids=list(range(NCORES)), trace=trace,
    )
    out = np.empty((N, S, L, E), np.float32)
    out[:] = bo_np  # masked query rows: attention output is 0, fc adds bo
    for n in range(NCORES):
        oT = np.asarray(res.results[n]["outT"], dtype=np.float32)  # (S,8,P,QP)
        for s in range(S):
            c = int(cnt[n, s])
            if c:
                out[n, s, order[n, s, :c], :] = (
                    oT[s].reshape(E, QP)[:, :c].T + bo_np)
    return out, res


def kernel(**inputs) -> np.ndarray:
    out, _ = run(inputs, trace=False)
    return out
